# revision 44
# baseline (speedup 1.0000x reference)
"""Askey-Wilson KAN layer forward on 8 TRN2 NeuronCores.

Math: y[b,o] = sum_{i,d} P_d(x[b,i]) * coeffs[i,o,d] where P_d satisfies a
3-term recurrence with *scalar* coefficients (functions of a,b,c,d,q only).
Hence P_d(x) = sum_k g[d,k] x^k with a tiny host-computable (9,9) matrix g,
and the whole layer collapses to 9 accumulated matmuls in the monomial basis:

    y = s0 + sum_{k=1..8} (x^k) @ W_k,
    W_k[i,o] = sum_d coeffs[i,o,d] * g[d,k],  s0[o] = sum_i W_0[i,o]

Device kernel (per core, batch-sharded 1024 rows): compute x^k tiles with 7
vector muls in f32 (rounded to bf16 for the matmul stationary operand on the
otherwise-idle Scalar/Vector engines), stream bf16 W_k from HBM, accumulate
64 matmuls (f32 PSUM) per [128b x 512o] PSUM tile, add s0 during the PSUM
drain. Data-parallel across 8 cores: no collectives. Measured ~242 us on HW
(pure matmul streaming floor is 1024 x 215.8 ns = 221 us + ~20 us of fixed
Tile prologue/epilogue), rel err vs the f32 reference ~1.8e-3.
"""

import sys
import types

import numpy as np

import concourse.bacc as bacc
import concourse.mybir as mybir
import concourse.tile as tile
from concourse.bass_utils import run_bass_kernel_spmd


def _ensure_axon_hooks_stub():
    """bass_utils imports antenv.axon_hooks when tracing is requested; some
    containers lack it. Install a no-op stub so a stray BASS_TRACE=1 in the
    environment degrades to no-trace instead of crashing."""
    try:
        import antenv.axon_hooks  # noqa: F401

        return
    except ImportError:
        pass
    try:
        import antenv
    except ImportError:
        return
    mod = types.ModuleType("antenv.axon_hooks")
    state = {"hook": None}
    mod.set_axon_ntff_profile_hook = lambda h: state.__setitem__("hook", h)
    mod.get_axon_ntff_profile_hook = lambda: state["hook"]
    sys.modules["antenv.axon_hooks"] = mod
    antenv.axon_hooks = mod


_ensure_axon_hooks_stub()

N_CORES = 8
B_FULL = 8192
I_DIM = 1024
O_DIM = 1024
DEG = 8
ND = DEG + 1  # 9 basis degrees
B_LOC = B_FULL // N_CORES  # 1024 batch rows per core

P = 128              # partitions
IC = I_DIM // P      # 8 contraction chunks
ON = 512             # output free-dim tile (one PSUM bank)
OC_TILES = O_DIM // ON  # 2
BT = B_LOC // P      # 8 batch tiles per core

F32 = mybir.dt.float32
F32R = mybir.dt.float32r
BF16 = mybir.dt.bfloat16

_COMPILED_NC = None
LAST_RESULT = None  # BassKernelResults of the most recent run (for profiling)
RUN_KWARGS = {}     # extra kwargs for run_bass_kernel_spmd (profiling)


def _monomial_transform(a, b, c, d, q):
    """g[d, k] with P_d(x) = sum_k g[d,k] x^k, computed in float64."""
    g = np.zeros((ND, ND), dtype=np.float64)
    g[0, 0] = 1.0
    den1 = 1.0 + a * b * c * d * q * q
    g[1, 1] = 2.0 * (1.0 + a * b * q) / den1
    g[1, 0] = -(a + b) * (1.0 + c * d * q) / den1
    for n in range(2, ND):
        An = (1 - a * b * q ** (n - 1)) * (1 - c * d * q ** (n - 1)) * (1 - a * b * c * d * q ** (2 * n - 2))
        An = An / ((1 - a * b * c * d * q ** (2 * n - 1)) * (1 - a * b * c * d * q ** (2 * n)))
        Cn = (1 - q ** n) * (1 - a * b * q ** (n - 1)) * (1 - c * d * q ** (n - 1)) * (1 - a * b * c * d * q ** (2 * n - 2))
        Cn = Cn / ((1 - a * b * c * d * q ** (2 * n - 2)) * (1 - a * b * c * d * q ** (2 * n - 1)))
        inv = 1.0 / (1.0 - q ** n)
        shifted = np.concatenate(([0.0], g[n - 1, :-1]))  # multiply by x
        g[n] = 2.0 * inv * shifted - An * inv * g[n - 1] - Cn * inv * g[n - 2]
    return g


def _build_kernel():
    nc = bacc.Bacc(
        "TRN2",
        target_bir_lowering=False,
        debug=False,
        enable_asserts=False,
        num_devices=N_CORES,
    )
    xT_h = nc.dram_tensor("xT", [I_DIM, B_LOC], F32, kind="ExternalInput")
    w_h = nc.dram_tensor("w", [DEG, I_DIM, O_DIM], BF16, kind="ExternalInput")
    s0_h = nc.dram_tensor("s0", [1, O_DIM], F32, kind="ExternalInput")
    out_h = nc.dram_tensor("out", [B_LOC, O_DIM], F32, kind="ExternalOutput")
    xT = xT_h.ap()
    w = w_h.ap()
    out = out_h.ap()

    with tile.TileContext(nc) as tc:
        with (
            tc.tile_pool(name="xt", bufs=1) as xpool,
            tc.tile_pool(name="bx", bufs=1) as bxpool,
            tc.tile_pool(name="s0p", bufs=1) as s0pool,
            tc.tile_pool(name="pow", bufs=2) as ppool,
            tc.tile_pool(name="bpw", bufs=3) as bppool,
            tc.tile_pool(name="wts", bufs=3) as wpool,
            tc.tile_pool(name="stage", bufs=2) as spool,
            tc.tile_pool(name="psum", bufs=8, space="PSUM") as psum_pool,
        ):
            # x^T resident, one tile per 128-row contraction chunk so each
            # k=1 matmul only waits for its own chunk's DMA. Interleave the
            # k=1 weight chunks so matmuls can start after one (x, w) pair.
            # The stationary matmul operand is a bf16 copy (made on the
            # otherwise-idle Scalar engine): 2-byte weights load ~2x faster
            # into the PE array, keeping LDWEIGHTS off the critical path.
            xts = []
            bxts = []
            w1ts = []
            for c in range(IC):
                xc = xpool.tile([P, B_LOC], F32, tag=f"x{c}", name=f"xt_{c}")
                nc.sync.dma_start(out=xc[:], in_=xT[c * P:(c + 1) * P, :])
                xts.append(xc)
                bxc = bxpool.tile([P, B_LOC], BF16, tag=f"bx{c}", name=f"bxt_{c}")
                nc.vector.tensor_copy(out=bxc[:], in_=xc[:])
                bxts.append(bxc)
                wc = wpool.tile([P, ON], BF16, tag=f"w{c}", name=f"w_0_1_{c}")
                nc.sync.dma_start(out=wc[:], in_=w[0, c * P:(c + 1) * P, 0:ON])
                w1ts.append(wc)

            s0t = s0pool.tile([P, O_DIM], F32)
            nc.sync.dma_start(out=s0t[:], in_=s0_h.ap().to_broadcast((P, O_DIM)))

            # Warm up the PE HAM clock gate on scratch data while the first
            # real DMAs are in flight (cold PE runs at 1.2 GHz for ~3.4us).
            scratch = s0pool.tile([P, ON + P], BF16, name="scratch")
            nc.gpsimd.memset(scratch[:], 1.0)

            for oc in range(OC_TILES):
                psums = [
                    psum_pool.tile([P, ON], F32, tag="ps", name=f"ps_{oc}_{i}")
                    for i in range(BT)
                ]
                if oc == 0:
                    for j in range(9):
                        nc.tensor.matmul(
                            psums[j % BT][:, :],
                            lhsT=scratch[:, ON:ON + P],
                            rhs=scratch[:, 0:ON],
                            start=True,
                            stop=True,
                        )
                prev_f = None
                for k in range(1, ND):
                    if k == 1:
                        cur = bxts
                        cur_f = xts
                    else:
                        cur = []
                        cur_f = []
                        for c in range(IC):
                            nt = ppool.tile(
                                [P, B_LOC], F32, tag=f"p{c}",
                                name=f"pow_{oc}_{k}_{c}",
                            )
                            nc.vector.tensor_mul(
                                out=nt[:], in0=prev_f[c][:], in1=xts[c][:]
                            )
                            bt16 = bppool.tile(
                                [P, B_LOC], BF16, tag=f"bp{c}",
                                name=f"bpow_{oc}_{k}_{c}",
                            )
                            if c < 2:
                                nc.vector.tensor_copy(out=bt16[:], in_=nt[:])
                            else:
                                nc.scalar.copy(out=bt16[:], in_=nt[:])
                            cur.append(bt16)
                            cur_f.append(nt)
                    if oc == 0 and k == 1:
                        wts = w1ts
                    else:
                        wts = []
                        for c in range(IC):
                            wc = wpool.tile(
                                [P, ON], BF16, tag=f"w{c}", name=f"w_{oc}_{k}_{c}"
                            )
                            nc.sync.dma_start(
                                out=wc[:],
                                in_=w[
                                    k - 1,
                                    c * P:(c + 1) * P,
                                    oc * ON:(oc + 1) * ON,
                                ],
                            )
                            wts.append(wc)
                    if k < DEG:
                        loop = [(ic, bt) for ic in range(IC) for bt in range(BT)]
                    else:
                        # last degree: finish banks one-by-one so the PSUM
                        # drains overlap the remaining matmuls
                        loop = [(ic, bt) for bt in range(BT) for ic in range(IC)]
                    for ic, bt in loop:
                        nc.tensor.matmul(
                            psums[bt][:, :],
                            lhsT=cur[ic][:, bt * P:(bt + 1) * P],
                            rhs=wts[ic][:],
                            start=(k == 1 and ic == 0),
                            stop=(k == DEG and ic == IC - 1),
                        )
                    prev_f = cur_f
                for bt in range(BT):
                    st = spool.tile([P, ON], F32, tag="stage", name=f"st_{oc}_{bt}")
                    nc.vector.tensor_add(
                        out=st[:],
                        in0=psums[bt][:],
                        in1=s0t[:, oc * ON:(oc + 1) * ON],
                    )
                    nc.sync.dma_start(
                        out=out[bt * P:(bt + 1) * P, oc * ON:(oc + 1) * ON],
                        in_=st[:],
                    )
    nc.compile()
    return nc


def _get_nc():
    global _COMPILED_NC
    if _COMPILED_NC is None:
        _COMPILED_NC = _build_kernel()
    return _COMPILED_NC


def kernel(x, a, b, c, d, q, coeffs):
    global LAST_RESULT
    x = np.asarray(x, dtype=np.float32)
    coeffs = np.asarray(coeffs)
    a0 = float(np.asarray(a).reshape(-1)[0])
    b0 = float(np.asarray(b).reshape(-1)[0])
    c0 = float(np.asarray(c).reshape(-1)[0])
    d0 = float(np.asarray(d).reshape(-1)[0])
    q0 = float(np.asarray(q).reshape(-1)[0])

    g = _monomial_transform(a0, b0, c0, d0, q0)  # [d, k]
    # W_k[i, o] = sum_d coeffs[i,o,d] g[d,k]  -> [k, i, o] contiguous
    wm = np.einsum(
        "iod,dk->kio", coeffs.astype(np.float64), g, optimize=True
    )
    import ml_dtypes

    s0 = np.ascontiguousarray(
        wm[0].sum(axis=0, keepdims=True).astype(np.float32)
    )  # [1, O]
    wk = np.ascontiguousarray(
        wm[1:].astype(np.float32).astype(ml_dtypes.bfloat16)
    )  # [8, I, O] bf16

    nc = _get_nc()
    in_maps = []
    for core in range(N_CORES):
        xs = x[core * B_LOC:(core + 1) * B_LOC, :]  # [B_LOC, I]
        xT = np.ascontiguousarray(xs.T)  # [I, B_LOC]
        in_maps.append({"xT": xT, "w": wk, "s0": s0})

    res = run_bass_kernel_spmd(
        nc, in_maps, core_ids=list(range(N_CORES)), **RUN_KWARGS
    )
    LAST_RESULT = res
    y = np.concatenate([res.results[i]["out"] for i in range(N_CORES)], axis=0)
    return np.ascontiguousarray(y.astype(np.float32))


# revision 45
# speedup vs baseline: 1.0010x; 1.0010x over previous
"""Askey-Wilson KAN layer forward on 8 TRN2 NeuronCores.

Math: y[b,o] = sum_{i,d} P_d(x[b,i]) * coeffs[i,o,d] where P_d satisfies a
3-term recurrence with *scalar* coefficients (functions of a,b,c,d,q only).
Hence P_d(x) = sum_k g[d,k] x^k with a tiny host-computable (9,9) matrix g,
and the whole layer collapses to 9 accumulated matmuls in the monomial basis:

    y = s0 + sum_{k=1..8} (x^k) @ W_k,
    W_k[i,o] = sum_d coeffs[i,o,d] * g[d,k],  s0[o] = sum_i W_0[i,o]

Device kernel (per core, batch-sharded 1024 rows): compute x^k tiles with 7
vector muls in f32 (rounded to bf16 for the matmul stationary operand on the
otherwise-idle Scalar/Vector engines), stream bf16 W_k from HBM, accumulate
64 matmuls (f32 PSUM) per [128b x 512o] PSUM tile, add s0 during the PSUM
drain. Data-parallel across 8 cores: no collectives. Measured ~242 us on HW
(pure matmul streaming floor is 1024 x 215.8 ns = 221 us + ~20 us of fixed
Tile prologue/epilogue), rel err vs the f32 reference ~1.8e-3.
"""

import sys
import types

import numpy as np

import concourse.bacc as bacc
import concourse.mybir as mybir
import concourse.tile as tile
from concourse.bass_utils import run_bass_kernel_spmd


def _ensure_axon_hooks_stub():
    """bass_utils imports antenv.axon_hooks when tracing is requested; some
    containers lack it. Install a no-op stub so a stray BASS_TRACE=1 in the
    environment degrades to no-trace instead of crashing."""
    try:
        import antenv.axon_hooks  # noqa: F401

        return
    except ImportError:
        pass
    try:
        import antenv
    except ImportError:
        return
    mod = types.ModuleType("antenv.axon_hooks")
    state = {"hook": None}
    mod.set_axon_ntff_profile_hook = lambda h: state.__setitem__("hook", h)
    mod.get_axon_ntff_profile_hook = lambda: state["hook"]
    sys.modules["antenv.axon_hooks"] = mod
    antenv.axon_hooks = mod


_ensure_axon_hooks_stub()

N_CORES = 8
B_FULL = 8192
I_DIM = 1024
O_DIM = 1024
DEG = 8
ND = DEG + 1  # 9 basis degrees
B_LOC = B_FULL // N_CORES  # 1024 batch rows per core

P = 128              # partitions
IC = I_DIM // P      # 8 contraction chunks
ON = 512             # output free-dim tile (one PSUM bank)
OC_TILES = O_DIM // ON  # 2
BT = B_LOC // P      # 8 batch tiles per core

F32 = mybir.dt.float32
F32R = mybir.dt.float32r
BF16 = mybir.dt.bfloat16

_COMPILED_NC = None
LAST_RESULT = None  # BassKernelResults of the most recent run (for profiling)
RUN_KWARGS = {}     # extra kwargs for run_bass_kernel_spmd (profiling)


def _monomial_transform(a, b, c, d, q):
    """g[d, k] with P_d(x) = sum_k g[d,k] x^k, computed in float64."""
    g = np.zeros((ND, ND), dtype=np.float64)
    g[0, 0] = 1.0
    den1 = 1.0 + a * b * c * d * q * q
    g[1, 1] = 2.0 * (1.0 + a * b * q) / den1
    g[1, 0] = -(a + b) * (1.0 + c * d * q) / den1
    for n in range(2, ND):
        An = (1 - a * b * q ** (n - 1)) * (1 - c * d * q ** (n - 1)) * (1 - a * b * c * d * q ** (2 * n - 2))
        An = An / ((1 - a * b * c * d * q ** (2 * n - 1)) * (1 - a * b * c * d * q ** (2 * n)))
        Cn = (1 - q ** n) * (1 - a * b * q ** (n - 1)) * (1 - c * d * q ** (n - 1)) * (1 - a * b * c * d * q ** (2 * n - 2))
        Cn = Cn / ((1 - a * b * c * d * q ** (2 * n - 2)) * (1 - a * b * c * d * q ** (2 * n - 1)))
        inv = 1.0 / (1.0 - q ** n)
        shifted = np.concatenate(([0.0], g[n - 1, :-1]))  # multiply by x
        g[n] = 2.0 * inv * shifted - An * inv * g[n - 1] - Cn * inv * g[n - 2]
    return g


def _build_kernel():
    nc = bacc.Bacc(
        "TRN2",
        target_bir_lowering=False,
        debug=False,
        enable_asserts=False,
        num_devices=N_CORES,
    )
    xT_h = nc.dram_tensor("xT", [I_DIM, B_LOC], F32, kind="ExternalInput")
    w_h = nc.dram_tensor("w", [DEG, I_DIM, O_DIM], BF16, kind="ExternalInput")
    s0_h = nc.dram_tensor("s0", [1, O_DIM], F32, kind="ExternalInput")
    out_h = nc.dram_tensor("out", [B_LOC, O_DIM], F32, kind="ExternalOutput")
    xT = xT_h.ap()
    w = w_h.ap()
    out = out_h.ap()

    with tile.TileContext(nc) as tc:
        with (
            tc.tile_pool(name="xt", bufs=1) as xpool,
            tc.tile_pool(name="bx", bufs=1) as bxpool,
            tc.tile_pool(name="s0p", bufs=1) as s0pool,
            tc.tile_pool(name="pow", bufs=2) as ppool,
            tc.tile_pool(name="bpw", bufs=3) as bppool,
            tc.tile_pool(name="wts", bufs=2) as wpool,
            tc.tile_pool(name="stage", bufs=2) as spool,
            tc.tile_pool(name="psum", bufs=8, space="PSUM") as psum_pool,
        ):
            # x^T resident, one tile per 128-row contraction chunk so each
            # k=1 matmul only waits for its own chunk's DMA. Interleave the
            # k=1 weight chunks so matmuls can start after one (x, w) pair.
            # The stationary matmul operand is a bf16 copy (made on the
            # otherwise-idle Scalar engine): 2-byte weights load ~2x faster
            # into the PE array, keeping LDWEIGHTS off the critical path.
            xts = []
            bxts = []
            w1ts = []
            for c in range(IC):
                xc = xpool.tile([P, B_LOC], F32, tag=f"x{c}", name=f"xt_{c}")
                nc.sync.dma_start(out=xc[:], in_=xT[c * P:(c + 1) * P, :])
                xts.append(xc)
                bxc = bxpool.tile([P, B_LOC], BF16, tag=f"bx{c}", name=f"bxt_{c}")
                nc.vector.tensor_copy(out=bxc[:], in_=xc[:])
                bxts.append(bxc)
                wc = wpool.tile([P, ON], BF16, tag=f"w{c}", name=f"w_0_1_{c}")
                nc.sync.dma_start(out=wc[:], in_=w[0, c * P:(c + 1) * P, 0:ON])
                w1ts.append(wc)

            s0t = s0pool.tile([P, O_DIM], F32)
            nc.sync.dma_start(out=s0t[:], in_=s0_h.ap().to_broadcast((P, O_DIM)))

            # Warm up the PE HAM clock gate on scratch data while the first
            # real DMAs are in flight (cold PE runs at 1.2 GHz for ~3.4us).
            scratch = s0pool.tile([P, ON + P], BF16, name="scratch")
            nc.gpsimd.memset(scratch[:], 1.0)

            for oc in range(OC_TILES):
                psums = [
                    psum_pool.tile([P, ON], F32, tag="ps", name=f"ps_{oc}_{i}")
                    for i in range(BT)
                ]
                if oc == 0:
                    for j in range(9):
                        nc.tensor.matmul(
                            psums[j % BT][:, :],
                            lhsT=scratch[:, ON:ON + P],
                            rhs=scratch[:, 0:ON],
                            start=True,
                            stop=True,
                        )
                prev_f = None
                for k in range(1, ND):
                    if k == 1:
                        cur = bxts
                        cur_f = xts
                    else:
                        cur = []
                        cur_f = []
                        for c in range(IC):
                            nt = ppool.tile(
                                [P, B_LOC], F32, tag=f"p{c}",
                                name=f"pow_{oc}_{k}_{c}",
                            )
                            nc.vector.tensor_mul(
                                out=nt[:], in0=prev_f[c][:], in1=xts[c][:]
                            )
                            bt16 = bppool.tile(
                                [P, B_LOC], BF16, tag=f"bp{c}",
                                name=f"bpow_{oc}_{k}_{c}",
                            )
                            if c < 2:
                                nc.vector.tensor_copy(out=bt16[:], in_=nt[:])
                            else:
                                nc.scalar.copy(out=bt16[:], in_=nt[:])
                            cur.append(bt16)
                            cur_f.append(nt)
                    if oc == 0 and k == 1:
                        wts = w1ts
                    else:
                        wts = []
                        for c in range(IC):
                            wc = wpool.tile(
                                [P, ON], BF16, tag=f"w{c}", name=f"w_{oc}_{k}_{c}"
                            )
                            nc.sync.dma_start(
                                out=wc[:],
                                in_=w[
                                    k - 1,
                                    c * P:(c + 1) * P,
                                    oc * ON:(oc + 1) * ON,
                                ],
                            )
                            wts.append(wc)
                    if k < DEG:
                        loop = [(ic, bt) for ic in range(IC) for bt in range(BT)]
                    else:
                        # last degree: finish banks one-by-one so the PSUM
                        # drains overlap the remaining matmuls
                        loop = [(ic, bt) for bt in range(BT) for ic in range(IC)]
                    for ic, bt in loop:
                        nc.tensor.matmul(
                            psums[bt][:, :],
                            lhsT=cur[ic][:, bt * P:(bt + 1) * P],
                            rhs=wts[ic][:],
                            start=(k == 1 and ic == 0),
                            stop=(k == DEG and ic == IC - 1),
                        )
                    prev_f = cur_f
                for bt in range(BT):
                    st = spool.tile([P, ON], F32, tag="stage", name=f"st_{oc}_{bt}")
                    nc.vector.tensor_add(
                        out=st[:],
                        in0=psums[bt][:],
                        in1=s0t[:, oc * ON:(oc + 1) * ON],
                    )
                    nc.sync.dma_start(
                        out=out[bt * P:(bt + 1) * P, oc * ON:(oc + 1) * ON],
                        in_=st[:],
                    )
    nc.compile()
    return nc


def _get_nc():
    global _COMPILED_NC
    if _COMPILED_NC is None:
        _COMPILED_NC = _build_kernel()
    return _COMPILED_NC


def kernel(x, a, b, c, d, q, coeffs):
    global LAST_RESULT
    x = np.asarray(x, dtype=np.float32)
    coeffs = np.asarray(coeffs)
    a0 = float(np.asarray(a).reshape(-1)[0])
    b0 = float(np.asarray(b).reshape(-1)[0])
    c0 = float(np.asarray(c).reshape(-1)[0])
    d0 = float(np.asarray(d).reshape(-1)[0])
    q0 = float(np.asarray(q).reshape(-1)[0])

    g = _monomial_transform(a0, b0, c0, d0, q0)  # [d, k]
    # W_k[i, o] = sum_d coeffs[i,o,d] g[d,k]  -> [k, i, o] contiguous
    wm = np.einsum(
        "iod,dk->kio", coeffs.astype(np.float64), g, optimize=True
    )
    import ml_dtypes

    s0 = np.ascontiguousarray(
        wm[0].sum(axis=0, keepdims=True).astype(np.float32)
    )  # [1, O]
    wk = np.ascontiguousarray(
        wm[1:].astype(np.float32).astype(ml_dtypes.bfloat16)
    )  # [8, I, O] bf16

    nc = _get_nc()
    in_maps = []
    for core in range(N_CORES):
        xs = x[core * B_LOC:(core + 1) * B_LOC, :]  # [B_LOC, I]
        xT = np.ascontiguousarray(xs.T)  # [I, B_LOC]
        in_maps.append({"xT": xT, "w": wk, "s0": s0})

    res = run_bass_kernel_spmd(
        nc, in_maps, core_ids=list(range(N_CORES)), **RUN_KWARGS
    )
    LAST_RESULT = res
    y = np.concatenate([res.results[i]["out"] for i in range(N_CORES)], axis=0)
    return np.ascontiguousarray(y.astype(np.float32))
